# revision 1
# baseline (speedup 1.0000x reference)
"""HOI relation-scoring kernel for Trainium2 (8 NeuronCores, data-parallel).

Full inputs in, full output out. Internally: batch dim (16 images) is
sharded 2-per-core across 8 cores; MLP weights are replicated.

Per-core pipeline (per image):
  1. Box fields are DMA'd (strided) into a [1, 96] row, cast to f32,
     1/area computed, and broadcast to all 128 partitions via a K=1
     ones-matmul -> bcast [128, 120] = (y1|y2|x1|x2|inv_area) x 24 boxes.
  2. A [128, 43, 24] f32 indicator mask (maskT[yx_chunk_row, chunk, box])
     is built with 7 DVE ops comparing host-constant flat-coordinate
     grids against the broadcast box fields.
  3. ROI sums: features[b] viewed [5476, 768] stream through the PE in
     43 chunks of [128, 768]; mask chunk [128, 24] is the stationary
     operand -> psum [24, 768] accumulates box sums. This makes the
     kernel HBM-bound (the features read is the roofline).
  4. Six PE transposes produce roiT [768(d) x 24(box)], scaled by
     inv_area during psum eviction -> ROI means, transposed layout.
  5. pairs@W1 is factorized: A.T = (hf@W1[:768]).T, B.T = (of@W1[768:]).T
     computed directly in [512, 24] orientation; pair expansion
     h1.T[d1, 16i+j] = A.T[d1, i] + b1 + B.T[d1, j] is a single fused
     scalar_tensor_tensor with broadcast APs; ReLU on ScalarE.
  6. Stages 2/3 chain in transposed layout (h1T as rhs, then h2T as
     lhsT) with no further transposes; biases are per-partition (b2) or
     a DMA-broadcast row tile (b3).
"""

import sys

import numpy as np

for _p in ("/opt/trn_rl_repo",):
    if _p not in sys.path:
        sys.path.insert(0, _p)

from contextlib import ExitStack

from concourse import bacc, mybir, tile
from concourse.bass import ts
from concourse.bass_utils import run_bass_kernel_spmd
from concourse.masks import make_identity

# Problem shapes (hardcoded per contract).
B, H, W, D = 16, 74, 74, 768
NH, NO = 8, 16
NB = NH + NO  # 24 boxes per image
NREL = 117
D1, D2 = 512, 256
NCORES = 8
BPC = B // NCORES  # images per core
YX = H * W  # 5476
NCHUNK = (YX + 127) // 128  # 43
TAIL = YX - 128 * (NCHUNK - 1)  # 100
NPAIR = NH * NO  # 128 pairs per image

F32 = mybir.dt.float32
I32 = mybir.dt.int32

_CACHE = {}


def _coord_consts():
    """Host constants: flat-index -> (y, x) coordinate grids, [128, NCHUNK].

    ycо[p, k] = (128*k + p) // W for valid flat indices, else -1e9 so all
    box compares fail and tail rows contribute zero.
    """
    flat = np.arange(NCHUNK * 128)
    valid = flat < YX
    y = np.where(valid, flat // W, -1e9).astype(np.float32)
    x = np.where(valid, flat % W, -1e9).astype(np.float32)
    yco = np.ascontiguousarray(y.reshape(NCHUNK, 128).T)
    xco = np.ascontiguousarray(x.reshape(NCHUNK, 128).T)
    return yco, xco


def _build_nc(repeat=1):
    import os

    variant = os.environ.get("KBENCH", "full")  # full | nomlp | dmaonly
    nc = bacc.Bacc("TRN2", target_bir_lowering=False)

    feats = nc.dram_tensor("feats", [BPC, H, W, D], F32, kind="ExternalInput")
    hbox = nc.dram_tensor("hbox", [BPC, NH, 4], I32, kind="ExternalInput")
    obox = nc.dram_tensor("obox", [BPC, NO, 4], I32, kind="ExternalInput")
    w1 = nc.dram_tensor("w1", [2 * D, D1], F32, kind="ExternalInput")
    b1 = nc.dram_tensor("b1", [D1], F32, kind="ExternalInput")
    w2 = nc.dram_tensor("w2", [D1, D2], F32, kind="ExternalInput")
    b2 = nc.dram_tensor("b2", [D2], F32, kind="ExternalInput")
    w3 = nc.dram_tensor("w3", [D2, NREL], F32, kind="ExternalInput")
    b3 = nc.dram_tensor("b3", [NREL], F32, kind="ExternalInput")
    yco = nc.dram_tensor("yco", [128, NCHUNK], F32, kind="ExternalInput")
    xco = nc.dram_tensor("xco", [128, NCHUNK], F32, kind="ExternalInput")
    out = nc.dram_tensor("out", [BPC * NPAIR, NREL], F32, kind="ExternalOutput")

    K1 = 2 * D // 128  # 12 chunks of W1 rows (first 6 = human half)
    K2 = D1 // 128  # 4 chunks of W2 rows
    K3 = D2 // 128  # 2 chunks of W3 rows
    MC1 = D1 // 128  # 4 output chunks of stage 1
    MC2 = D2 // 128  # 2 output chunks of stage 2
    DCH = D // 128  # 6 chunks of the feature dim

    with tile.TileContext(nc) as tc, ExitStack() as ctx:
        const = ctx.enter_context(tc.tile_pool(name="const", bufs=1))
        fpool = ctx.enter_context(tc.tile_pool(name="fpool", bufs=8))
        mpool = ctx.enter_context(tc.tile_pool(name="mpool", bufs=2))
        spool = ctx.enter_context(tc.tile_pool(name="spool", bufs=2))
        roi_ps = ctx.enter_context(tc.tile_pool(name="roi_ps", bufs=2, space="PSUM"))
        ppool = ctx.enter_context(tc.tile_pool(name="ppool", bufs=3, space="PSUM"))

        # ---------------- preamble: weights + constants ----------------
        w1sb = []
        w1v = w1[:].rearrange("(c p) m -> c p m", p=128)
        for c in range(K1):
            t = const.tile([128, D1], F32, tag=f"w1_{c}")
            nc.sync.dma_start(t[:], w1v[c])
            w1sb.append(t)
        w2sb = []
        w2v = w2[:].rearrange("(c p) m -> c p m", p=128)
        for c in range(K2):
            t = const.tile([128, D2], F32, tag=f"w2_{c}")
            nc.sync.dma_start(t[:], w2v[c])
            w2sb.append(t)
        w3sb = []
        w3v = w3[:].rearrange("(c p) m -> c p m", p=128)
        for c in range(K3):
            t = const.tile([128, NREL], F32, tag=f"w3_{c}")
            nc.sync.dma_start(t[:], w3v[c])
            w3sb.append(t)

        b1sb = []
        for mc in range(MC1):
            t = const.tile([128, 1], F32, tag=f"b1_{mc}")
            nc.sync.dma_start(t[:], b1[ts(mc, 128)][:, None])
            b1sb.append(t)
        b2sb = []
        for mc in range(MC2):
            t = const.tile([128, 1], F32, tag=f"b2_{mc}")
            nc.sync.dma_start(t[:], b2[ts(mc, 128)][:, None])
            b2sb.append(t)
        b3bc = const.tile([128, NREL], F32, tag="b3bc")
        nc.sync.dma_start(b3bc[:], b3[None, :].to_broadcast((128, NREL)))

        ycosb = const.tile([128, NCHUNK], F32, tag="ycosb")
        nc.sync.dma_start(ycosb[:], yco[:])
        xcosb = const.tile([128, NCHUNK], F32, tag="xcosb")
        nc.sync.dma_start(xcosb[:], xco[:])

        ident = const.tile([128, 128], F32, tag="ident")
        make_identity(nc, ident[:])
        ones_row = const.tile([1, 128], F32, tag="ones_row")
        nc.vector.memset(ones_row[:], 1.0)

        for b in [i % BPC for i in range(BPC * repeat)]:
            fv = feats[b].rearrange("h w d -> (h w) d")  # [5476, 768]

            # ---- box fields -> [1, 120] row: y1 | y2 | x1 | x2 | 1/area
            boxi = spool.tile([1, 4 * NB], I32, tag="boxi")
            for fi, col in enumerate((1, 3, 0, 2)):  # (x1,y1,x2,y2) -> y1,y2,x1,x2
                nc.sync.dma_start(
                    boxi[:, fi * NB : fi * NB + NH], hbox[b, :, col][None, :]
                )
                nc.sync.dma_start(
                    boxi[:, fi * NB + NH : (fi + 1) * NB], obox[b, :, col][None, :]
                )
            boxf = spool.tile([1, 5 * NB], F32, tag="boxf")
            nc.vector.tensor_copy(boxf[:, 0 : 4 * NB], boxi[:])
            dy = spool.tile([1, NB], F32, tag="dy")
            nc.vector.tensor_sub(dy[:], boxf[:, NB : 2 * NB], boxf[:, 0:NB])
            dx = spool.tile([1, NB], F32, tag="dx")
            nc.vector.tensor_sub(dx[:], boxf[:, 3 * NB : 4 * NB], boxf[:, 2 * NB : 3 * NB])
            nc.vector.tensor_mul(dy[:], dy[:], dx[:])
            nc.vector.reciprocal(boxf[:, 4 * NB : 5 * NB], dy[:])

            # ---- broadcast the 120 fields to all partitions (K=1 matmul)
            bps = ppool.tile([128, 5 * NB], F32, tag="pp")
            nc.tensor.matmul(bps[:], ones_row[:], boxf[:], start=True, stop=True)
            bcast = spool.tile([128, 5 * NB], F32, tag="bcast")
            nc.scalar.copy(bcast[:], bps[:])

            # ---- indicator mask [128, NCHUNK, NB]
            mask = mpool.tile([128, NCHUNK, NB], F32, tag="mask")
            mtmp = mpool.tile([128, NCHUNK, NB], F32, tag="mtmp")
            shp = (128, NCHUNK, NB)
            yv = ycosb[:].unsqueeze(2).to_broadcast(shp)
            xv = xcosb[:].unsqueeze(2).to_broadcast(shp)
            y1v = bcast[:, 0:NB].unsqueeze(1).to_broadcast(shp)
            y2v = bcast[:, NB : 2 * NB].unsqueeze(1).to_broadcast(shp)
            x1v = bcast[:, 2 * NB : 3 * NB].unsqueeze(1).to_broadcast(shp)
            x2v = bcast[:, 3 * NB : 4 * NB].unsqueeze(1).to_broadcast(shp)
            nc.vector.tensor_tensor(mask[:], yv, y1v, mybir.AluOpType.is_ge)
            nc.vector.tensor_tensor(mtmp[:], yv, y2v, mybir.AluOpType.is_lt)
            nc.vector.tensor_mul(mask[:], mask[:], mtmp[:])
            nc.vector.tensor_tensor(mtmp[:], xv, x1v, mybir.AluOpType.is_ge)
            nc.vector.tensor_mul(mask[:], mask[:], mtmp[:])
            nc.vector.tensor_tensor(mtmp[:], xv, x2v, mybir.AluOpType.is_lt)
            nc.vector.tensor_mul(mask[:], mask[:], mtmp[:])

            # ---- ROI sums: stream features, mask stationary
            pa = roi_ps.tile([NB, 512], F32, tag="roiA")
            pb = roi_ps.tile([NB, 256], F32, tag="roiB")
            for k in range(NCHUNK):
                rows = 128 if k < NCHUNK - 1 else TAIL
                ft = fpool.tile([128, D], F32, tag="feat")
                nc.sync.dma_start(ft[:rows, :], fv[k * 128 : k * 128 + rows, :])
                if variant == "dmaonly":
                    continue
                lhs = mask[:rows, k, :]
                nc.tensor.matmul(
                    pa[:], lhs, ft[:rows, 0:512], start=(k == 0), stop=(k == NCHUNK - 1)
                )
                nc.tensor.matmul(
                    pb[:], lhs, ft[:rows, 512:768], start=(k == 0), stop=(k == NCHUNK - 1)
                )
            if variant == "dmaonly":
                continue
            roi = spool.tile([NB, D], F32, tag="roi")
            nc.vector.tensor_copy(roi[:, 0:512], pa[:])
            nc.vector.tensor_copy(roi[:, 512:768], pb[:])

            # ---- transpose to roiT [128, DCH, NB], fold in 1/area
            roit = spool.tile([128, DCH, NB], F32, tag="roit")
            for t6 in range(DCH):
                pt = ppool.tile([128, NB], F32, tag="pp")
                nc.tensor.transpose(pt[:], roi[:, ts(t6, 128)], ident[:NB, :NB])
                nc.vector.tensor_mul(
                    roit[:, t6, :], pt[:], bcast[:, 4 * NB : 5 * NB]
                )

            # ---- stage 1: A.T | B.T -> pair-expand -> relu -> h1T
            h1sb = []
            for mc in range(MC1):
                p1 = ppool.tile([128, NB], F32, tag="pp")
                for kc in range(DCH):
                    nc.tensor.matmul(
                        p1[:, 0:NH],
                        w1sb[kc][:, ts(mc, 128)],
                        roit[:, kc, 0:NH],
                        start=(kc == 0),
                        stop=(kc == DCH - 1),
                    )
                for kc in range(DCH):
                    nc.tensor.matmul(
                        p1[:, NH:NB],
                        w1sb[DCH + kc][:, ts(mc, 128)],
                        roit[:, kc, NH:NB],
                        start=(kc == 0),
                        stop=(kc == DCH - 1),
                    )
                ab = spool.tile([128, NB], F32, tag="ab")
                nc.scalar.copy(ab[:], p1[:])
                pre = spool.tile([128, NH, NO], F32, tag="pre")
                nc.vector.scalar_tensor_tensor(
                    pre[:],
                    ab[:, 0:NH].unsqueeze(2).to_broadcast((128, NH, NO)),
                    b1sb[mc][:],
                    ab[:, NH:NB].unsqueeze(1).to_broadcast((128, NH, NO)),
                    mybir.AluOpType.add,
                    mybir.AluOpType.add,
                )
                h1 = spool.tile([128, NPAIR], F32, tag=f"h1_{mc}")
                nc.scalar.activation(h1[:], pre[:], mybir.ActivationFunctionType.Relu)
                h1sb.append(h1)

            # ---- stage 2: h2T[m2] = relu(W2[:, m2].T @ h1 + b2)
            h2sb = []
            for m2 in range(MC2):
                p2 = ppool.tile([128, NPAIR], F32, tag="pp")
                for kc in range(K2):
                    nc.tensor.matmul(
                        p2[:],
                        w2sb[kc][:, ts(m2, 128)],
                        h1sb[kc][:],
                        start=(kc == 0),
                        stop=(kc == K2 - 1),
                    )
                h2 = spool.tile([128, NPAIR], F32, tag=f"h2_{m2}")
                nc.scalar.activation(
                    h2[:], p2[:], mybir.ActivationFunctionType.Relu, bias=b2sb[m2][:]
                )
                h2sb.append(h2)

            # ---- stage 3: out = h2 @ W3 + b3
            p3 = ppool.tile([NPAIR, NREL], F32, tag="pp")
            for kc in range(K3):
                nc.tensor.matmul(
                    p3[:], h2sb[kc][:], w3sb[kc][:], start=(kc == 0), stop=(kc == K3 - 1)
                )
            osb = spool.tile([NPAIR, NREL], F32, tag="osb")
            nc.vector.tensor_add(osb[:], p3[:], b3bc[:])
            nc.sync.dma_start(out[ts(b, NPAIR), :], osb[:])

    nc.compile()
    return nc


def _get_nc(repeat=1):
    key = f"nc{repeat}"
    if key not in _CACHE:
        _CACHE[key] = _build_nc(repeat)
    return _CACHE[key]


def _in_maps(inputs):
    feats = np.ascontiguousarray(np.asarray(inputs["features"], dtype=np.float32))
    hb = np.ascontiguousarray(np.asarray(inputs["human_boxes"], dtype=np.int32))
    ob = np.ascontiguousarray(np.asarray(inputs["obj_boxes"], dtype=np.int32))
    yco, xco = _coord_consts()
    common = {
        "w1": np.ascontiguousarray(np.asarray(inputs["W1"], dtype=np.float32)),
        "b1": np.ascontiguousarray(np.asarray(inputs["b1"], dtype=np.float32)),
        "w2": np.ascontiguousarray(np.asarray(inputs["W2"], dtype=np.float32)),
        "b2": np.ascontiguousarray(np.asarray(inputs["b2"], dtype=np.float32)),
        "w3": np.ascontiguousarray(np.asarray(inputs["W3"], dtype=np.float32)),
        "b3": np.ascontiguousarray(np.asarray(inputs["b3"], dtype=np.float32)),
        "yco": yco,
        "xco": xco,
    }
    maps = []
    for c in range(NCORES):
        m = dict(common)
        m["feats"] = np.ascontiguousarray(feats[c * BPC : (c + 1) * BPC])
        m["hbox"] = np.ascontiguousarray(hb[c * BPC : (c + 1) * BPC])
        m["obox"] = np.ascontiguousarray(ob[c * BPC : (c + 1) * BPC])
        maps.append(m)
    return maps


def run(trace=False, **inputs):
    nc = _get_nc()
    res = run_bass_kernel_spmd(nc, _in_maps(inputs), list(range(NCORES)), trace=trace)
    out = np.concatenate([res.results[c]["out"] for c in range(NCORES)], axis=0)
    return out.astype(np.float32), res


def timed_run(iters=20, repeat=1, **inputs):
    """Mirror bass2jax.run_bass_via_pjrt's 8-core shard_map path, but stage
    inputs on device once and time repeated executions. Returns
    (full_output, best_wall_ns) where best_wall_ns = min over iters of one
    sharded dispatch (upper bound on per-core HW exec time)."""
    import time

    import jax
    from jax.sharding import Mesh, PartitionSpec
    from jax.experimental.shard_map import shard_map

    from concourse import bass2jax, mybir as _mybir

    nc = _get_nc(repeat)
    in_maps = _in_maps(inputs)
    n_cores = NCORES

    partition_name = nc.partition_id_tensor.name if nc.partition_id_tensor else None
    in_names, out_names, out_avals, zero_outs = [], [], [], []
    for alloc in nc.m.functions[0].allocations:
        if not isinstance(alloc, _mybir.MemoryLocationSet):
            continue
        name = alloc.memorylocations[0].name
        if alloc.kind == "ExternalInput":
            if name != partition_name:
                in_names.append(name)
        elif alloc.kind == "ExternalOutput":
            shape = tuple(alloc.tensor_shape)
            dtype = _mybir.dt.np(alloc.dtype)
            out_names.append(name)
            out_avals.append(jax.core.ShapedArray(shape, dtype))
            zero_outs.append(np.zeros(shape, dtype))
    n_params = len(in_names)
    n_outs = len(out_avals)
    all_in_names = list(in_names) + list(out_names)
    if partition_name is not None:
        all_in_names.append(partition_name)
    donate = tuple(range(n_params, n_params + n_outs))

    def _body(*args):
        operands = list(args)
        if partition_name is not None:
            operands.append(bass2jax.partition_id_tensor())
        outs = bass2jax._bass_exec_p.bind(
            *operands,
            out_avals=tuple(out_avals),
            in_names=tuple(all_in_names),
            out_names=tuple(out_names),
            lowering_input_output_aliases=(),
            sim_require_finite=True,
            sim_require_nnan=True,
            nc=nc,
        )
        return tuple(outs)

    bass2jax.install_neuronx_cc_hook()
    devices = jax.devices()[:n_cores]
    mesh = Mesh(np.asarray(devices), ("core",))
    in_specs = (PartitionSpec("core"),) * (n_params + n_outs)
    out_specs = (PartitionSpec("core"),) * len(out_names)
    sharded = jax.jit(
        shard_map(_body, mesh=mesh, in_specs=in_specs, out_specs=out_specs,
                  check_rep=False),
        donate_argnums=donate,
        keep_unused=True,
    )
    per_core = [[np.asarray(m[name]) for name in in_names] for m in in_maps]
    concat_in = [
        np.concatenate([per_core[c][i] for c in range(n_cores)], axis=0)
        for i in range(n_params)
    ]
    concat_zeros = [
        np.zeros((n_cores * z.shape[0], *z.shape[1:]), z.dtype) for z in zero_outs
    ]
    sharding = jax.sharding.NamedSharding(mesh, PartitionSpec("core"))
    dev_in = [jax.device_put(a, sharding) for a in concat_in]
    out_arrs = None
    best = None
    for _ in range(iters):
        dev_zeros = [jax.device_put(z, sharding) for z in concat_zeros]
        jax.block_until_ready(dev_zeros)
        t0 = time.perf_counter()
        res = sharded(*dev_in, *dev_zeros)
        jax.block_until_ready(res)
        dt = time.perf_counter() - t0
        if best is None or dt < best:
            best = dt
            out_arrs = res
    outs = [
        np.asarray(out_arrs[i]).reshape(n_cores, *out_avals[i].shape)
        for i in range(n_outs)
    ]
    full = np.concatenate([outs[out_names.index("out")][c] for c in range(n_cores)], 0)
    return full.astype(np.float32), int(best * 1e9)


def kernel(**inputs):
    out, _ = run(trace=False, **inputs)
    return out



# revision 39
# speedup vs baseline: 629.5595x; 629.5595x over previous
"""HOI relation-scoring kernel for Trainium2 (8 NeuronCores, data-parallel).

Full inputs in, full output out. Internally: batch dim (16 images) is
sharded 2-per-core across 8 cores; MLP weights are replicated.

Per-core pipeline (heavily DMA-overlapped; target is the HBM roofline of
~34MB of features per core):
  1. Head: boxes for BOTH images land as 2 contiguous DMAs; per-field
     gather is done on-chip (strided DVE copies — strided 16B-stride box
     DMAs proved unreliable). Box fields are broadcast to 128 partitions
     via a K=1 ones-matmul, and both images' [128, 43, 24] inclusion
     masks are built upfront by 7 DVE compare/mul ops each, landing in an
     fp32r tile.
  2. Features stream as GROUPED DMAs ([128, 6, 768] per descriptor set,
     7 groups + 1 tail chunk per image) — few dma_start instructions, so
     the issuing engine is never the throttle. Even groups ride the SP
     (sync) HWDGE ring; odd groups ride the Activation (scalar) ring
     (KRING=single puts all on SP). Weights/biases/coords/outputs are
     kept off the feature rings' hot path (scalar ring, ordered so they
     never block feature descriptors).
  3. ROI sums: mask chunk [rows, 24] is the PE-stationary operand; f32r
     feature chunks stream through (fp32r = 1 cycle/column for free dims
     >= 256 vs 4 for fp32; rel-err budget 2e-2 dwarfs the ~1e-4 cost).
     PSUM [24, 512] + [24, 256] accumulate over 43 chunks per image.
  4. Six PE transposes produce roiT [768(d) x 24(box)], scaled by
     1/area during psum eviction -> ROI means in transposed layout.
  5. pairs@W1 is factorized: A.T = (hf@W1[:768]).T, B.T = (of@W1[768:]).T
     computed directly in [512, 24] orientation; pair expansion
     h1.T[d1, 16i+j] = A.T[d1, i] + b1 + B.T[d1, j] is a single fused
     scalar_tensor_tensor; ReLU via DVE tensor_scalar max (the Activation
     engine is busy issuing DMAs).
  6. Stages 2/3 chain in transposed layout with no further transposes;
     stage-2 bias+ReLU is one fused DVE tensor_scalar op.
"""

import os
import sys

import numpy as np

for _p in ("/opt/trn_rl_repo",):
    if _p not in sys.path:
        sys.path.insert(0, _p)

from contextlib import ExitStack

from concourse import bacc, mybir, tile
from concourse.bass import ts
from concourse.bass_utils import run_bass_kernel_spmd
from concourse.masks import make_identity

# Problem shapes (hardcoded per contract).
B, H, W, D = 16, 74, 74, 768
NH, NO = 8, 16
NB = NH + NO  # 24 boxes per image
NREL = 117
D1, D2 = 512, 256
NCORES = 8
BPC = B // NCORES  # images per core
YX = H * W  # 5476
NCHUNK = (YX + 127) // 128  # 43
TAIL = YX - 128 * (NCHUNK - 1)  # 100
NPAIR = NH * NO  # 128 pairs per image
G = 2  # feature chunks per grouped DMA (small -> the rings pipeline well)
NG = (NCHUNK - 1) // G  # 21 full groups (42 chunks); chunk 42 is the tail

F32 = mybir.dt.float32
F32R = mybir.dt.float32r
I32 = mybir.dt.int32

_CACHE = {}


def _coord_consts():
    """Host constants: flat-index -> (y, x) coordinate grids, [NCHUNK, 128].

    yco[k, p] = (128*k + p) // W for valid flat indices, else -1e9 so all
    box compares fail and tail rows contribute zero. Shipped in (k, p)
    layout (43 contiguous 512B rows -> one cheap DMA) and transposed to
    [128, NCHUNK] on-chip via the PE.
    """
    flat = np.arange(NCHUNK * 128)
    valid = flat < YX
    y = np.where(valid, flat // W, -1e9).astype(np.float32)
    x = np.where(valid, flat % W, -1e9).astype(np.float32)
    yco = np.ascontiguousarray(y.reshape(NCHUNK, 128))
    xco = np.ascontiguousarray(x.reshape(NCHUNK, 128))
    return yco, xco


def _build_nc():
    variant = os.environ.get("KBENCH", "full")  # full | dmaonly
    ring = os.environ.get("KRING", "dual")  # dual | single

    nc = bacc.Bacc("TRN2", target_bir_lowering=False)

    feats = nc.dram_tensor("feats", [BPC, H, W, D], F32, kind="ExternalInput")
    hbox = nc.dram_tensor("hbox", [BPC, NH, 4], I32, kind="ExternalInput")
    obox = nc.dram_tensor("obox", [BPC, NO, 4], I32, kind="ExternalInput")
    w1 = nc.dram_tensor("w1", [2 * D, D1], F32, kind="ExternalInput")
    b1 = nc.dram_tensor("b1", [D1], F32, kind="ExternalInput")
    w2 = nc.dram_tensor("w2", [D1, D2], F32, kind="ExternalInput")
    b2 = nc.dram_tensor("b2", [D2], F32, kind="ExternalInput")
    w3 = nc.dram_tensor("w3", [D2, NREL], F32, kind="ExternalInput")
    b3 = nc.dram_tensor("b3", [NREL], F32, kind="ExternalInput")
    yco = nc.dram_tensor("yco", [NCHUNK, 128], F32, kind="ExternalInput")
    xco = nc.dram_tensor("xco", [NCHUNK, 128], F32, kind="ExternalInput")
    out = nc.dram_tensor("out", [BPC * NPAIR, NREL], F32, kind="ExternalOutput")

    K1 = 2 * D // 128  # 12 chunks of W1 rows (first 6 = human half)
    K2 = D1 // 128  # 4 chunks of W2 rows
    K3 = D2 // 128  # 2 chunks of W3 rows
    MC1 = D1 // 128  # 4 output chunks of stage 1
    MC2 = D2 // 128  # 2 output chunks of stage 2
    DCH = D // 128  # 6 chunks of the feature dim

    with tile.TileContext(nc) as tc, ExitStack() as ctx:
        const = ctx.enter_context(tc.tile_pool(name="const", bufs=1))
        fe_pool = ctx.enter_context(tc.tile_pool(name="fe", bufs=10))
        fo_pool = ctx.enter_context(tc.tile_pool(name="fo", bufs=11))
        ft_pool = ctx.enter_context(tc.tile_pool(name="ftl", bufs=2))
        mpool = ctx.enter_context(tc.tile_pool(name="mpool", bufs=2))
        spool = ctx.enter_context(tc.tile_pool(name="spool", bufs=2))
        roi_ps = ctx.enter_context(tc.tile_pool(name="roi_ps", bufs=2, space="PSUM"))
        ppool = ctx.enter_context(tc.tile_pool(name="ppool", bufs=2, space="PSUM"))
        p3pool = ctx.enter_context(tc.tile_pool(name="p3pool", bufs=1, space="PSUM"))

        # ---------------- head: coords/boxes/biases on the gpsimd ring ----
        # The two HWDGE rings carry ONLY features + weights; everything
        # small rides software DGE so no 128-segment descriptor storm ever
        # sits in front of feature data. Column-per-partition layouts
        # (coords [128,43], per-partition biases) are produced by PE
        # transposes from cheap contiguous row-major DMAs.
        # boxes + coords ride the FRONT of the sync HWDGE ring (tiny, fast);
        # biases ride software DGE (not needed until ~60us in).
        boxi = const.tile([1, BPC, NB, 4], I32, tag="boxi")
        nc.sync.dma_start(boxi[:, :, 0:NH, :], hbox[:][None, :, :, :])
        nc.sync.dma_start(boxi[:, :, NH:NB, :], obox[:][None, :, :, :])
        ycot = const.tile([NCHUNK, 128], F32, tag="ycot")
        nc.scalar.dma_start(ycot[:], yco[:])
        xcot = const.tile([NCHUNK, 128], F32, tag="xcot")
        nc.scalar.dma_start(xcot[:], xco[:])

        ident = const.tile([128, 128], F32, tag="ident")
        make_identity(nc, ident[:])

        b12 = const.tile([MC1 + MC2, 128], F32, tag="b12")
        nc.gpsimd.dma_start(b12[0:MC1, :], b1[:].rearrange("(c p) -> c p", p=128))
        nc.gpsimd.dma_start(
            b12[MC1 : MC1 + MC2, :], b2[:].rearrange("(c p) -> c p", p=128)
        )
        b3row = const.tile([1, NREL], F32, tag="b3row")
        nc.gpsimd.dma_start(b3row[:], b3[None, :])

        ones_row = const.tile([1, 128], F32, tag="ones_row")
        nc.vector.memset(ones_row[:], 1.0)

        # coords land [43,128] f32, transposed on the PE, evicted as bf16
        # (values <= 74 are bf16-exact) so the mask compares run all-bf16
        # at double DVE rate.
        BF16 = mybir.dt.bfloat16
        ycosb = const.tile([128, NCHUNK], BF16, tag="ycosb")
        xcosb = const.tile([128, NCHUNK], BF16, tag="xcosb")
        b12t = const.tile([128, MC1 + MC2], F32, tag="b12t")
        for src, dst, n in ((ycot, ycosb, NCHUNK), (xcot, xcosb, NCHUNK),
                            (b12, b12t, MC1 + MC2)):
            pt = ppool.tile([128, n], F32, tag="pp")
            nc.tensor.transpose(pt[:], src[:], ident[:n, :n])
            nc.vector.tensor_copy(dst[:], pt[:])

        # ---------------- upfront per-image box math + masks --------------
        invars, maskrs = [], []
        for b in range(BPC):
            # box fields -> [1, 120] row: y1 | y2 | x1 | x2 | 1/area
            boxf = spool.tile([1, 5 * NB], F32, tag="boxf")
            for fi, col in enumerate((1, 3, 0, 2)):  # -> y1,y2,x1,x2
                nc.vector.tensor_copy(
                    boxf[:, fi * NB : (fi + 1) * NB], boxi[:, b, :, col]
                )
            dy = spool.tile([1, NB], F32, tag="dy")
            nc.vector.tensor_sub(dy[:], boxf[:, NB : 2 * NB], boxf[:, 0:NB])
            dx = spool.tile([1, NB], F32, tag="dx")
            nc.vector.tensor_sub(
                dx[:], boxf[:, 3 * NB : 4 * NB], boxf[:, 2 * NB : 3 * NB]
            )
            nc.vector.tensor_mul(dy[:], dy[:], dx[:])
            nc.vector.reciprocal(boxf[:, 4 * NB : 5 * NB], dy[:])

            # broadcast the 120 fields to all partitions (K=1 matmul); the
            # mask compares read the psum tile directly, only 1/area is
            # evicted (it is needed much later, at ROI eviction time).
            bps = ppool.tile([128, 5 * NB], F32, tag="pp")
            nc.tensor.matmul(bps[:], ones_row[:], boxf[:], start=True, stop=True)
            invar = spool.tile([128, NB], F32, tag="invar")
            nc.vector.tensor_copy(invar[:], bps[:, 4 * NB : 5 * NB])
            invars.append(invar)
            bc16 = spool.tile([128, 4 * NB], BF16, tag="bc16")
            nc.vector.tensor_copy(bc16[:], bps[:, 0 : 4 * NB])

            # indicator mask [128, NCHUNK, NB]: all-bf16 compares (2x DVE
            # rate; box coords <= 74 are bf16-exact, values are 0/1) with
            # the final product landing in the fp32r tile the ROI matmuls
            # consume.
            mask = mpool.tile([128, NCHUNK, NB], BF16, tag="mask")
            mtmp = mpool.tile([128, NCHUNK, NB], BF16, tag="mtmp")
            maskr = mpool.tile([128, NCHUNK, NB], F32R, tag="maskr")
            shp = (128, NCHUNK, NB)
            yv = ycosb[:].unsqueeze(2).to_broadcast(shp)
            xv = xcosb[:].unsqueeze(2).to_broadcast(shp)
            y1v = bc16[:, 0:NB].unsqueeze(1).to_broadcast(shp)
            y2v = bc16[:, NB : 2 * NB].unsqueeze(1).to_broadcast(shp)
            x1v = bc16[:, 2 * NB : 3 * NB].unsqueeze(1).to_broadcast(shp)
            x2v = bc16[:, 3 * NB : 4 * NB].unsqueeze(1).to_broadcast(shp)
            nc.vector.tensor_tensor(mask[:], yv, y1v, mybir.AluOpType.is_ge)
            nc.vector.tensor_tensor(mtmp[:], yv, y2v, mybir.AluOpType.is_lt)
            nc.vector.tensor_mul(mask[:], mask[:], mtmp[:])
            nc.vector.tensor_tensor(mtmp[:], xv, x1v, mybir.AluOpType.is_ge)
            nc.vector.tensor_mul(mask[:], mask[:], mtmp[:])
            nc.vector.tensor_tensor(mtmp[:], xv, x2v, mybir.AluOpType.is_lt)
            nc.vector.tensor_mul(maskr[:], mask[:], mtmp[:])
            maskrs.append(maskr)

        # ---------------- feature DMA issue (both images) -----------------
        # Even groups on the SP ring, odd groups on the scalar ring (dual).
        # Weights/biases go on the scalar ring between image 0's and image
        # 1's odd groups: early enough for stage 1, never blocking features.
        gtiles = [[None] * NG for _ in range(BPC)]
        ttiles = [None] * BPC

        def issue_features(b):
            fv = feats[b].rearrange("h w d -> (h w) d")  # [5476, 768]
            for g in range(NG):
                src = fv[g * G * 128 : (g + 1) * G * 128, :].rearrange(
                    "(j p) d -> p j d", p=128
                ).bitcast(F32R)
                if ring == "dual" and g % 2 == 0:
                    t = fo_pool.tile([128, G, D], F32R, tag="fgo")
                    nc.scalar.dma_start(t[:], src)
                else:
                    t = fe_pool.tile([128, G, D], F32R, tag="fge")
                    nc.sync.dma_start(t[:], src)
                gtiles[b][g] = t
            t = ft_pool.tile([128, D], F32R, tag="ftail")
            nc.sync.dma_start(
                t[:TAIL, :], fv[NG * G * 128 : YX, :].bitcast(F32R)
            )
            ttiles[b] = t

        def issue_weights():
            # between the two images' feature blocks, split across rings
            nonlocal w1sb, w2sb, w3sb
            w1sb = []
            w1v = w1[:].rearrange("(c p) m -> c p m", p=128)
            for c in range(K1):
                # f32r: stage 1 streams W1 through the PE against the
                # roit stationary at 1 cycle/column.
                t = const.tile([128, D1], F32R, tag=f"w1_{c}")
                eng = nc.sync if c < K1 // 2 else nc.scalar
                eng.dma_start(t[:], w1v[c].bitcast(F32R))
                w1sb.append(t)
            w2sb = []
            w2v = w2[:].rearrange("(c p) m -> c p m", p=128)
            for c in range(K2):
                t = const.tile([128, D2], F32, tag=f"w2_{c}")
                nc.sync.dma_start(t[:], w2v[c])
                w2sb.append(t)
            w3sb = []
            w3v = w3[:].rearrange("(c p) m -> c p m", p=128)
            for c in range(K3):
                t = const.tile([128, NREL], F32, tag=f"w3_{c}")
                nc.sync.dma_start(t[:], w3v[c])
                w3sb.append(t)

        w1sb = w2sb = w3sb = None
        issue_features(0)
        issue_weights()
        issue_features(1)

        # ---------------- compute per image -------------------------------
        osbs = []
        for b in range(BPC):
            maskr, invar = maskrs[b], invars[b]
            pa = roi_ps.tile([NB, 512], F32, tag="roiA")
            pb = roi_ps.tile([NB, 256], F32, tag="roiB")
            if variant == "dmaonly":
                continue
            for k in range(NCHUNK):
                rows = 128 if k < NCHUNK - 1 else TAIL
                if k < NG * G:
                    ft = gtiles[b][k // G][:, k % G, :]
                else:
                    ft = ttiles[b][:]
                lhs = maskr[:rows, k, :]
                nc.tensor.matmul(
                    pa[:], lhs, ft[:rows, 0:512], start=(k == 0), stop=(k == NCHUNK - 1)
                )
                nc.tensor.matmul(
                    pb[:], lhs, ft[:rows, 512:768], start=(k == 0), stop=(k == NCHUNK - 1)
                )
            roi = spool.tile([NB, D], F32, tag="roi")
            nc.vector.tensor_copy(roi[:, 0:512], pa[:])
            nc.vector.tensor_copy(roi[:, 512:768], pb[:])

            # transpose to roiT [128, DCH, NB] (f32r), fold in 1/area
            roit = spool.tile([128, DCH, NB], F32R, tag="roit")
            for t6 in range(DCH):
                pt = ppool.tile([128, NB], F32, tag="pp")
                nc.tensor.transpose(pt[:], roi[:, ts(t6, 128)], ident[:NB, :NB])
                nc.vector.tensor_mul(roit[:, t6, :], pt[:], invar[:])

            # stage 1: A = hf@W1h [8, 512], B = of@W1o [16, 512] with the
            # small roit chunk stationary and W1 streaming at 1 cyc/col —
            # 12 wide matmuls instead of 48 tiny ones. Then 8 cheap PE
            # transposes put A.T/B.T chunks in [d1, box] orientation and a
            # fused DVE op pair-expands + biases + (separate op) ReLUs.
            pA = roi_ps.tile([NH, D1], F32, tag="roiA")
            pB = roi_ps.tile([NO, D1], F32, tag="roiB")
            for kc in range(DCH):
                nc.tensor.matmul(
                    pA[:],
                    roit[:, kc, 0:NH],
                    w1sb[kc][:],
                    start=(kc == 0),
                    stop=(kc == DCH - 1),
                )
                nc.tensor.matmul(
                    pB[:],
                    roit[:, kc, NH:NB],
                    w1sb[DCH + kc][:],
                    start=(kc == 0),
                    stop=(kc == DCH - 1),
                )
            asb = spool.tile([NH, D1], F32, tag="asb")
            nc.vector.tensor_copy(asb[:], pA[:])
            bsb = spool.tile([NO, D1], F32, tag="bsb")
            nc.vector.tensor_copy(bsb[:], pB[:])
            h1sb = []
            for mc in range(MC1):
                pat = ppool.tile([128, NH], F32, tag="pp")
                nc.tensor.transpose(pat[:], asb[:, ts(mc, 128)], ident[:NH, :NH])
                pbt = ppool.tile([128, NO], F32, tag="pp")
                nc.tensor.transpose(pbt[:], bsb[:, ts(mc, 128)], ident[:NO, :NO])
                # DVE may read only one non-scalar PSUM input: evict pat
                patsb = spool.tile([128, NH], F32, tag="patsb")
                nc.vector.tensor_copy(patsb[:], pat[:])
                pre = spool.tile([128, NH, NO], F32, tag="pre")
                nc.vector.scalar_tensor_tensor(
                    pre[:],
                    patsb[:].unsqueeze(2).to_broadcast((128, NH, NO)),
                    b12t[:, mc : mc + 1],
                    pbt[:].unsqueeze(1).to_broadcast((128, NH, NO)),
                    mybir.AluOpType.add,
                    mybir.AluOpType.add,
                )
                h1 = spool.tile([128, NPAIR], F32, tag=f"h1_{mc}")
                nc.vector.tensor_scalar(
                    h1[:],
                    pre[:].rearrange("p a b -> p (a b)"),
                    0.0,
                    None,
                    mybir.AluOpType.max,
                )
                h1sb.append(h1)

            # stage 2: h2T[m2] = relu(W2[:, m2].T @ h1 + b2)  (fused DVE op)
            h2sb = []
            for m2 in range(MC2):
                p2 = ppool.tile([128, NPAIR], F32, tag="pp")
                for kc in range(K2):
                    nc.tensor.matmul(
                        p2[:],
                        w2sb[kc][:, ts(m2, 128)],
                        h1sb[kc][:],
                        start=(kc == 0),
                        stop=(kc == K2 - 1),
                    )
                h2 = spool.tile([128, NPAIR], F32, tag=f"h2_{m2}")
                nc.vector.tensor_scalar(
                    h2[:], p2[:], b12t[:, MC1 + m2 : MC1 + m2 + 1], 0.0,
                    mybir.AluOpType.add, mybir.AluOpType.max,
                )
                h2sb.append(h2)

            # stage 3: out = h2 @ W3 + b3 (bias folded in as a K=1 matmul)
            p3 = p3pool.tile([NPAIR, NREL], F32, tag="pp3")
            for kc in range(K3):
                nc.tensor.matmul(
                    p3[:], h2sb[kc][:], w3sb[kc][:], start=(kc == 0), stop=False
                )
            nc.tensor.matmul(p3[:], ones_row[:], b3row[:], start=False, stop=True)
            osb = spool.tile([NPAIR, NREL], F32, tag="osb")
            nc.vector.tensor_copy(osb[:], p3[:])
            osbs.append(osb)

        # output DMAs last so they never gate feature descriptors
        for b, osb in enumerate(osbs):
            eng = nc.sync if b == 0 else nc.scalar
            eng.dma_start(out[ts(b, NPAIR), :], osb[:])

    nc.compile()
    return nc


def _get_nc():
    key = (os.environ.get("KBENCH", "full"), os.environ.get("KRING", "dual"))
    if key not in _CACHE:
        _CACHE[key] = _build_nc()
    return _CACHE[key]


def _in_maps(inputs):
    feats = np.ascontiguousarray(np.asarray(inputs["features"], dtype=np.float32))
    hb = np.ascontiguousarray(np.asarray(inputs["human_boxes"], dtype=np.int32))
    ob = np.ascontiguousarray(np.asarray(inputs["obj_boxes"], dtype=np.int32))
    yco, xco = _coord_consts()
    common = {
        "w1": np.ascontiguousarray(np.asarray(inputs["W1"], dtype=np.float32)),
        "b1": np.ascontiguousarray(np.asarray(inputs["b1"], dtype=np.float32)),
        "w2": np.ascontiguousarray(np.asarray(inputs["W2"], dtype=np.float32)),
        "b2": np.ascontiguousarray(np.asarray(inputs["b2"], dtype=np.float32)),
        "w3": np.ascontiguousarray(np.asarray(inputs["W3"], dtype=np.float32)),
        "b3": np.ascontiguousarray(np.asarray(inputs["b3"], dtype=np.float32)),
        "yco": yco,
        "xco": xco,
    }
    maps = []
    for c in range(NCORES):
        m = dict(common)
        m["feats"] = np.ascontiguousarray(feats[c * BPC : (c + 1) * BPC])
        m["hbox"] = np.ascontiguousarray(hb[c * BPC : (c + 1) * BPC])
        m["obox"] = np.ascontiguousarray(ob[c * BPC : (c + 1) * BPC])
        maps.append(m)
    return maps


def run(trace=False, **inputs):
    nc = _get_nc()
    res = run_bass_kernel_spmd(nc, _in_maps(inputs), list(range(NCORES)), trace=trace)
    out = np.concatenate([res.results[c]["out"] for c in range(NCORES)], axis=0)
    return out.astype(np.float32), res


def kernel(**inputs):
    out, _ = run(trace=False, **inputs)
    return out


# revision 42
# speedup vs baseline: 651.7679x; 1.0353x over previous
"""HOI relation-scoring kernel for Trainium2 (8 NeuronCores, data-parallel).

Full inputs in, full output out. Internally: batch dim (16 images) is
sharded 2-per-core across 8 cores; MLP weights are replicated.

Per-core pipeline (heavily DMA-overlapped; target is the HBM roofline of
~34MB of features per core):
  1. Head: boxes for BOTH images land as 2 contiguous DMAs; per-field
     gather is done on-chip (strided DVE copies — strided 16B-stride box
     DMAs proved unreliable). Box fields are broadcast to 128 partitions
     via a K=1 ones-matmul, and both images' [128, 43, 24] inclusion
     masks are built upfront by 7 DVE compare/mul ops each, landing in an
     fp32r tile.
  2. Features stream as GROUPED DMAs ([128, 6, 768] per descriptor set,
     7 groups + 1 tail chunk per image) — few dma_start instructions, so
     the issuing engine is never the throttle. Even groups ride the SP
     (sync) HWDGE ring; odd groups ride the Activation (scalar) ring
     (KRING=single puts all on SP). Weights/biases/coords/outputs are
     kept off the feature rings' hot path (scalar ring, ordered so they
     never block feature descriptors).
  3. ROI sums: mask chunk [rows, 24] is the PE-stationary operand; f32r
     feature chunks stream through (fp32r = 1 cycle/column for free dims
     >= 256 vs 4 for fp32; rel-err budget 2e-2 dwarfs the ~1e-4 cost).
     PSUM [24, 512] + [24, 256] accumulate over 43 chunks per image.
  4. Six PE transposes produce roiT [768(d) x 24(box)], scaled by
     1/area during psum eviction -> ROI means in transposed layout.
  5. pairs@W1 is factorized: A.T = (hf@W1[:768]).T, B.T = (of@W1[768:]).T
     computed directly in [512, 24] orientation; pair expansion
     h1.T[d1, 16i+j] = A.T[d1, i] + b1 + B.T[d1, j] is a single fused
     scalar_tensor_tensor; ReLU via DVE tensor_scalar max (the Activation
     engine is busy issuing DMAs).
  6. Stages 2/3 chain in transposed layout with no further transposes;
     stage-2 bias+ReLU is one fused DVE tensor_scalar op.
"""

import os
import sys

import numpy as np

for _p in ("/opt/trn_rl_repo",):
    if _p not in sys.path:
        sys.path.insert(0, _p)

from contextlib import ExitStack

from concourse import bacc, mybir, tile
from concourse.bass import ts
from concourse.bass_utils import run_bass_kernel_spmd
from concourse.masks import make_identity

# Problem shapes (hardcoded per contract).
B, H, W, D = 16, 74, 74, 768
NH, NO = 8, 16
NB = NH + NO  # 24 boxes per image
NREL = 117
D1, D2 = 512, 256
NCORES = 8
BPC = B // NCORES  # images per core
YX = H * W  # 5476
NCHUNK = (YX + 127) // 128  # 43
TAIL = YX - 128 * (NCHUNK - 1)  # 100
NPAIR = NH * NO  # 128 pairs per image
G = 2  # feature chunks per grouped DMA (small -> the rings pipeline well)
NG = (NCHUNK - 1) // G  # 21 full groups (42 chunks); chunk 42 is the tail

F32 = mybir.dt.float32
F32R = mybir.dt.float32r
I32 = mybir.dt.int32

_CACHE = {}


def _coord_consts():
    """Host constants: flat-index -> (y, x) coordinate grids, [NCHUNK, 128].

    yco[k, p] = (128*k + p) // W for valid flat indices, else -1e9 so all
    box compares fail and tail rows contribute zero. Shipped in (k, p)
    layout (43 contiguous 512B rows -> one cheap DMA) and transposed to
    [128, NCHUNK] on-chip via the PE.
    """
    flat = np.arange(NCHUNK * 128)
    valid = flat < YX
    y = np.where(valid, flat // W, -1e9).astype(np.float32)
    x = np.where(valid, flat % W, -1e9).astype(np.float32)
    yco = np.ascontiguousarray(y.reshape(NCHUNK, 128))
    xco = np.ascontiguousarray(x.reshape(NCHUNK, 128))
    return yco, xco


def _build_nc():
    variant = os.environ.get("KBENCH", "full")  # full | dmaonly
    ring = os.environ.get("KRING", "dual")  # dual | single

    nc = bacc.Bacc("TRN2", target_bir_lowering=False)

    feats = nc.dram_tensor("feats", [BPC, H, W, D], F32, kind="ExternalInput")
    hbox = nc.dram_tensor("hbox", [BPC, NH, 4], I32, kind="ExternalInput")
    obox = nc.dram_tensor("obox", [BPC, NO, 4], I32, kind="ExternalInput")
    w1 = nc.dram_tensor("w1", [2 * D, D1], F32, kind="ExternalInput")
    b1 = nc.dram_tensor("b1", [D1], F32, kind="ExternalInput")
    w2 = nc.dram_tensor("w2", [D1, D2], F32, kind="ExternalInput")
    b2 = nc.dram_tensor("b2", [D2], F32, kind="ExternalInput")
    w3 = nc.dram_tensor("w3", [D2, NREL], F32, kind="ExternalInput")
    b3 = nc.dram_tensor("b3", [NREL], F32, kind="ExternalInput")
    yco = nc.dram_tensor("yco", [NCHUNK, 128], F32, kind="ExternalInput")
    xco = nc.dram_tensor("xco", [NCHUNK, 128], F32, kind="ExternalInput")
    out = nc.dram_tensor("out", [BPC * NPAIR, NREL], F32, kind="ExternalOutput")

    K1 = 2 * D // 128  # 12 chunks of W1 rows (first 6 = human half)
    K2 = D1 // 128  # 4 chunks of W2 rows
    K3 = D2 // 128  # 2 chunks of W3 rows
    MC1 = D1 // 128  # 4 output chunks of stage 1
    MC2 = D2 // 128  # 2 output chunks of stage 2
    DCH = D // 128  # 6 chunks of the feature dim

    with tile.TileContext(nc) as tc, ExitStack() as ctx:
        const = ctx.enter_context(tc.tile_pool(name="const", bufs=1))
        fe_pool = ctx.enter_context(tc.tile_pool(name="fe", bufs=10))
        fo_pool = ctx.enter_context(tc.tile_pool(name="fo", bufs=11))
        ft_pool = ctx.enter_context(tc.tile_pool(name="ftl", bufs=2))
        mpool = ctx.enter_context(tc.tile_pool(name="mpool", bufs=2))
        spool = ctx.enter_context(tc.tile_pool(name="spool", bufs=2))
        roi_ps = ctx.enter_context(tc.tile_pool(name="roi_ps", bufs=2, space="PSUM"))
        ppool = ctx.enter_context(tc.tile_pool(name="ppool", bufs=2, space="PSUM"))
        p3pool = ctx.enter_context(tc.tile_pool(name="p3pool", bufs=1, space="PSUM"))

        # ---------------- head: coords/boxes/biases on the gpsimd ring ----
        # The two HWDGE rings carry ONLY features + weights; everything
        # small rides software DGE so no 128-segment descriptor storm ever
        # sits in front of feature data. Column-per-partition layouts
        # (coords [128,43], per-partition biases) are produced by PE
        # transposes from cheap contiguous row-major DMAs.
        # boxes + coords ride the FRONT of the sync HWDGE ring (tiny, fast);
        # biases ride software DGE (not needed until ~60us in).
        boxi = const.tile([1, BPC, NB, 4], I32, tag="boxi")
        nc.sync.dma_start(boxi[:, :, 0:NH, :], hbox[:][None, :, :, :])
        nc.sync.dma_start(boxi[:, :, NH:NB, :], obox[:][None, :, :, :])
        yxcot = const.tile([2 * NCHUNK, 128], F32, tag="yxcot")
        nc.scalar.dma_start(yxcot[0:NCHUNK, :], yco[:])
        nc.scalar.dma_start(yxcot[NCHUNK : 2 * NCHUNK, :], xco[:])
        b12 = const.tile([MC1 + MC2, 128], F32, tag="b12")
        nc.scalar.dma_start(b12[0:MC1, :], b1[:].rearrange("(c p) -> c p", p=128))
        nc.scalar.dma_start(
            b12[MC1 : MC1 + MC2, :], b2[:].rearrange("(c p) -> c p", p=128)
        )
        b3row = const.tile([1, NREL], F32, tag="b3row")
        nc.scalar.dma_start(b3row[:], b3[None, :])

        ident = const.tile([128, 128], F32, tag="ident")
        make_identity(nc, ident[:])

        ones_row = const.tile([1, 128], F32, tag="ones_row")
        nc.vector.memset(ones_row[:], 1.0)

        # coords land [2*43,128] f32, transposed on the PE in one shot,
        # evicted as bf16 (values <= 74 are bf16-exact) so the mask
        # compares run all-bf16.
        BF16 = mybir.dt.bfloat16
        yxcosb = const.tile([128, 2 * NCHUNK], BF16, tag="yxcosb")
        ptc = ppool.tile([128, 2 * NCHUNK], F32, tag="pp")
        nc.tensor.transpose(ptc[:], yxcot[:], ident[: 2 * NCHUNK, : 2 * NCHUNK])
        nc.vector.tensor_copy(yxcosb[:], ptc[:])
        ycosb = yxcosb[:, 0:NCHUNK]
        xcosb = yxcosb[:, NCHUNK : 2 * NCHUNK]
        b12t = const.tile([128, MC1 + MC2], F32, tag="b12t")
        ptb = ppool.tile([128, MC1 + MC2], F32, tag="pp")
        nc.tensor.transpose(ptb[:], b12[:], ident[: MC1 + MC2, : MC1 + MC2])
        nc.vector.tensor_copy(b12t[:], ptb[:])

        # ---------------- upfront per-image box math + masks --------------
        invars, maskrs = [], []
        for b in range(BPC):
            # box fields -> [1, 120] row: y1 | y2 | x1 | x2 | 1/area
            boxf = spool.tile([1, 5 * NB], F32, tag="boxf")
            for fi, col in enumerate((1, 3, 0, 2)):  # -> y1,y2,x1,x2
                nc.vector.tensor_copy(
                    boxf[:, fi * NB : (fi + 1) * NB], boxi[:, b, :, col]
                )
            dy = spool.tile([1, NB], F32, tag="dy")
            nc.vector.tensor_sub(dy[:], boxf[:, NB : 2 * NB], boxf[:, 0:NB])
            dx = spool.tile([1, NB], F32, tag="dx")
            nc.vector.tensor_sub(
                dx[:], boxf[:, 3 * NB : 4 * NB], boxf[:, 2 * NB : 3 * NB]
            )
            nc.vector.tensor_mul(dy[:], dy[:], dx[:])
            nc.vector.reciprocal(boxf[:, 4 * NB : 5 * NB], dy[:])

            # broadcast the 120 fields to all partitions (K=1 matmul); the
            # mask compares read the psum tile directly, only 1/area is
            # evicted (it is needed much later, at ROI eviction time).
            bps = ppool.tile([128, 5 * NB], F32, tag="pp")
            nc.tensor.matmul(bps[:], ones_row[:], boxf[:], start=True, stop=True)
            invar = spool.tile([128, NB], F32, tag="invar")
            nc.vector.tensor_copy(invar[:], bps[:, 4 * NB : 5 * NB])
            invars.append(invar)
            bc16 = spool.tile([128, 4 * NB], BF16, tag="bc16")
            nc.vector.tensor_copy(bc16[:], bps[:, 0 : 4 * NB])

            # indicator mask [128, NCHUNK, NB]: all-bf16 compares (2x DVE
            # rate; box coords <= 74 are bf16-exact, values are 0/1) with
            # the final product landing in the fp32r tile the ROI matmuls
            # consume.
            mask = mpool.tile([128, NCHUNK, NB], BF16, tag="mask")
            mtmp = mpool.tile([128, NCHUNK, NB], BF16, tag="mtmp")
            maskr = mpool.tile([128, NCHUNK, NB], F32R, tag="maskr")
            shp = (128, NCHUNK, NB)
            yv = ycosb.unsqueeze(2).to_broadcast(shp)
            xv = xcosb.unsqueeze(2).to_broadcast(shp)
            y1v = bc16[:, 0:NB].unsqueeze(1).to_broadcast(shp)
            y2v = bc16[:, NB : 2 * NB].unsqueeze(1).to_broadcast(shp)
            x1v = bc16[:, 2 * NB : 3 * NB].unsqueeze(1).to_broadcast(shp)
            x2v = bc16[:, 3 * NB : 4 * NB].unsqueeze(1).to_broadcast(shp)
            nc.vector.tensor_tensor(mask[:], yv, y1v, mybir.AluOpType.is_ge)
            nc.vector.tensor_tensor(mtmp[:], yv, y2v, mybir.AluOpType.is_lt)
            nc.vector.tensor_mul(mask[:], mask[:], mtmp[:])
            nc.vector.tensor_tensor(mtmp[:], xv, x1v, mybir.AluOpType.is_ge)
            nc.vector.tensor_mul(mask[:], mask[:], mtmp[:])
            nc.vector.tensor_tensor(mtmp[:], xv, x2v, mybir.AluOpType.is_lt)
            nc.vector.tensor_mul(maskr[:], mask[:], mtmp[:])
            maskrs.append(maskr)

        # ---------------- feature DMA issue (both images) -----------------
        # Even groups on the SP ring, odd groups on the scalar ring (dual).
        # Weights/biases go on the scalar ring between image 0's and image
        # 1's odd groups: early enough for stage 1, never blocking features.
        gtiles = [[None] * NG for _ in range(BPC)]
        ttiles = [None] * BPC

        def issue_features(b):
            fv = feats[b].rearrange("h w d -> (h w) d")  # [5476, 768]
            for g in range(NG):
                src = fv[g * G * 128 : (g + 1) * G * 128, :].rearrange(
                    "(j p) d -> p j d", p=128
                ).bitcast(F32R)
                if ring == "dual" and g % 2 == 0:
                    t = fo_pool.tile([128, G, D], F32R, tag="fgo")
                    nc.scalar.dma_start(t[:], src)
                else:
                    t = fe_pool.tile([128, G, D], F32R, tag="fge")
                    nc.sync.dma_start(t[:], src)
                gtiles[b][g] = t
            t = ft_pool.tile([128, D], F32R, tag="ftail")
            nc.sync.dma_start(
                t[:TAIL, :], fv[NG * G * 128 : YX, :].bitcast(F32R)
            )
            ttiles[b] = t

        def issue_weights():
            # between the two images' feature blocks, split across rings
            nonlocal w1sb, w2sb, w3sb
            w1sb = []
            w1v = w1[:].rearrange("(c p) m -> c p m", p=128)
            for c in range(K1):
                # f32r: stage 1 streams W1 through the PE against the
                # roit stationary at 1 cycle/column.
                t = const.tile([128, D1], F32R, tag=f"w1_{c}")
                eng = nc.sync if c < K1 // 2 else nc.scalar
                eng.dma_start(t[:], w1v[c].bitcast(F32R))
                w1sb.append(t)
            w2sb = []
            w2v = w2[:].rearrange("(c p) m -> c p m", p=128)
            for c in range(K2):
                t = const.tile([128, D2], F32, tag=f"w2_{c}")
                nc.sync.dma_start(t[:], w2v[c])
                w2sb.append(t)
            w3sb = []
            w3v = w3[:].rearrange("(c p) m -> c p m", p=128)
            for c in range(K3):
                t = const.tile([128, NREL], F32, tag=f"w3_{c}")
                nc.sync.dma_start(t[:], w3v[c])
                w3sb.append(t)

        w1sb = w2sb = w3sb = None
        issue_features(0)
        issue_weights()
        issue_features(1)

        # ---------------- compute per image -------------------------------
        osbs = []
        for b in range(BPC):
            maskr, invar = maskrs[b], invars[b]
            pa = roi_ps.tile([NB, 512], F32, tag="roiA")
            pb = roi_ps.tile([NB, 256], F32, tag="roiB")
            if variant == "dmaonly":
                continue
            for k in range(NCHUNK):
                rows = 128 if k < NCHUNK - 1 else TAIL
                if k < NG * G:
                    ft = gtiles[b][k // G][:, k % G, :]
                else:
                    ft = ttiles[b][:]
                lhs = maskr[:rows, k, :]
                nc.tensor.matmul(
                    pa[:], lhs, ft[:rows, 0:512], start=(k == 0), stop=(k == NCHUNK - 1)
                )
                nc.tensor.matmul(
                    pb[:], lhs, ft[:rows, 512:768], start=(k == 0), stop=(k == NCHUNK - 1)
                )
            roi = spool.tile([NB, D], F32, tag="roi")
            nc.vector.tensor_copy(roi[:, 0:512], pa[:])
            nc.vector.tensor_copy(roi[:, 512:768], pb[:])

            # transpose to roiT [128, DCH, NB] (f32r), fold in 1/area
            roit = spool.tile([128, DCH, NB], F32R, tag="roit")
            for t6 in range(DCH):
                pt = ppool.tile([128, NB], F32, tag="pp")
                nc.tensor.transpose(pt[:], roi[:, ts(t6, 128)], ident[:NB, :NB])
                nc.vector.tensor_mul(roit[:, t6, :], pt[:], invar[:])

            # stage 1: A = hf@W1h [8, 512], B = of@W1o [16, 512] with the
            # small roit chunk stationary and W1 streaming at 1 cyc/col —
            # 12 wide matmuls instead of 48 tiny ones. Then 8 cheap PE
            # transposes put A.T/B.T chunks in [d1, box] orientation and a
            # fused DVE op pair-expands + biases + (separate op) ReLUs.
            pA = roi_ps.tile([NH, D1], F32, tag="roiA")
            pB = roi_ps.tile([NO, D1], F32, tag="roiB")
            for kc in range(DCH):
                nc.tensor.matmul(
                    pA[:],
                    roit[:, kc, 0:NH],
                    w1sb[kc][:],
                    start=(kc == 0),
                    stop=(kc == DCH - 1),
                )
                nc.tensor.matmul(
                    pB[:],
                    roit[:, kc, NH:NB],
                    w1sb[DCH + kc][:],
                    start=(kc == 0),
                    stop=(kc == DCH - 1),
                )
            asb = spool.tile([NH, D1], F32, tag="asb")
            nc.vector.tensor_copy(asb[:], pA[:])
            bsb = spool.tile([NO, D1], F32, tag="bsb")
            nc.vector.tensor_copy(bsb[:], pB[:])
            h1sb = []
            for mc in range(MC1):
                pat = ppool.tile([128, NH], F32, tag="pp")
                nc.tensor.transpose(pat[:], asb[:, ts(mc, 128)], ident[:NH, :NH])
                pbt = ppool.tile([128, NO], F32, tag="pp")
                nc.tensor.transpose(pbt[:], bsb[:, ts(mc, 128)], ident[:NO, :NO])
                # DVE may read only one non-scalar PSUM input: evict pat
                patsb = spool.tile([128, NH], F32, tag="patsb")
                nc.vector.tensor_copy(patsb[:], pat[:])
                pre = spool.tile([128, NH, NO], F32, tag="pre")
                nc.vector.scalar_tensor_tensor(
                    pre[:],
                    patsb[:].unsqueeze(2).to_broadcast((128, NH, NO)),
                    b12t[:, mc : mc + 1],
                    pbt[:].unsqueeze(1).to_broadcast((128, NH, NO)),
                    mybir.AluOpType.add,
                    mybir.AluOpType.add,
                )
                h1 = spool.tile([128, NPAIR], F32, tag=f"h1_{mc}")
                nc.vector.tensor_scalar(
                    h1[:],
                    pre[:].rearrange("p a b -> p (a b)"),
                    0.0,
                    None,
                    mybir.AluOpType.max,
                )
                h1sb.append(h1)

            # stage 2: h2T[m2] = relu(W2[:, m2].T @ h1 + b2)  (fused DVE op)
            h2sb = []
            for m2 in range(MC2):
                p2 = ppool.tile([128, NPAIR], F32, tag="pp")
                for kc in range(K2):
                    nc.tensor.matmul(
                        p2[:],
                        w2sb[kc][:, ts(m2, 128)],
                        h1sb[kc][:],
                        start=(kc == 0),
                        stop=(kc == K2 - 1),
                    )
                h2 = spool.tile([128, NPAIR], F32, tag=f"h2_{m2}")
                nc.vector.tensor_scalar(
                    h2[:], p2[:], b12t[:, MC1 + m2 : MC1 + m2 + 1], 0.0,
                    mybir.AluOpType.add, mybir.AluOpType.max,
                )
                h2sb.append(h2)

            # stage 3: out = h2 @ W3 + b3 (bias folded in as a K=1 matmul)
            p3 = p3pool.tile([NPAIR, NREL], F32, tag="pp3")
            for kc in range(K3):
                nc.tensor.matmul(
                    p3[:], h2sb[kc][:], w3sb[kc][:], start=(kc == 0), stop=False
                )
            nc.tensor.matmul(p3[:], ones_row[:], b3row[:], start=False, stop=True)
            osb = spool.tile([NPAIR, NREL], F32, tag="osb")
            nc.vector.tensor_copy(osb[:], p3[:])
            osbs.append(osb)

        # output DMAs last so they never gate feature descriptors
        for b, osb in enumerate(osbs):
            eng = nc.sync if b == 0 else nc.scalar
            eng.dma_start(out[ts(b, NPAIR), :], osb[:])

    nc.compile()
    return nc


def _get_nc():
    key = (os.environ.get("KBENCH", "full"), os.environ.get("KRING", "dual"))
    if key not in _CACHE:
        _CACHE[key] = _build_nc()
    return _CACHE[key]


def _in_maps(inputs):
    feats = np.ascontiguousarray(np.asarray(inputs["features"], dtype=np.float32))
    hb = np.ascontiguousarray(np.asarray(inputs["human_boxes"], dtype=np.int32))
    ob = np.ascontiguousarray(np.asarray(inputs["obj_boxes"], dtype=np.int32))
    yco, xco = _coord_consts()
    common = {
        "w1": np.ascontiguousarray(np.asarray(inputs["W1"], dtype=np.float32)),
        "b1": np.ascontiguousarray(np.asarray(inputs["b1"], dtype=np.float32)),
        "w2": np.ascontiguousarray(np.asarray(inputs["W2"], dtype=np.float32)),
        "b2": np.ascontiguousarray(np.asarray(inputs["b2"], dtype=np.float32)),
        "w3": np.ascontiguousarray(np.asarray(inputs["W3"], dtype=np.float32)),
        "b3": np.ascontiguousarray(np.asarray(inputs["b3"], dtype=np.float32)),
        "yco": yco,
        "xco": xco,
    }
    maps = []
    for c in range(NCORES):
        m = dict(common)
        m["feats"] = np.ascontiguousarray(feats[c * BPC : (c + 1) * BPC])
        m["hbox"] = np.ascontiguousarray(hb[c * BPC : (c + 1) * BPC])
        m["obox"] = np.ascontiguousarray(ob[c * BPC : (c + 1) * BPC])
        maps.append(m)
    return maps


def run(trace=False, **inputs):
    nc = _get_nc()
    res = run_bass_kernel_spmd(nc, _in_maps(inputs), list(range(NCORES)), trace=trace)
    out = np.concatenate([res.results[c]["out"] for c in range(NCORES)], axis=0)
    return out.astype(np.float32), res


def kernel(**inputs):
    out, _ = run(trace=False, **inputs)
    return out
